# revision 15
# baseline (speedup 1.0000x reference)
"""Trainium2 Bass kernel for nn_GCMKGATCL_Ablation (GNN message passing).

Self-contained: takes FULL unsharded inputs, shards rows across 8 NeuronCores,
runs a Bass/Tile SPMD kernel (fp32 everywhere on the selection path), returns
the full outputs matching reference().

Sharding: node dim N=8000 padded to 8192, 1024 rows per core. Dense adjacency
A^T slices per core; per-layer AllGather of side^T and of e. Top-16 per row via
chunked max8 + threshold-masked dense softmax-matmul aggregation (no dynamic
gather — indirect DMA unsupported in this environment).
"""

import numpy as np

NCORES = 8
P = 128
D = 128
N_USERS, N_ITEMS, N_ENT = 2500, 3000, 2500
N = N_USERS + N_ITEMS + N_ENT          # 8000
NPAD = 8192
RPD = NPAD // NCORES                   # 1024 rows per device
MT = RPD // P                          # 8 m-tiles per device
NCH = 16                               # score chunks
CH = N // NCH                          # 500 cols per chunk
NKT = (N + P - 1) // P                 # 63 agg contraction tiles (62*128+64)
AKT = NPAD // P                        # 64 side contraction tiles
L = 2
GATE_T = 0.5
BIG = 1.0e30

_PROGRAM_CACHE = {}


def _build_program():
    import concourse.bacc as bacc
    import concourse.bass as bass
    import concourse.mybir as mybir
    from concourse.tile import TileContext

    f32 = mybir.dt.float32
    AF = mybir.ActivationFunctionType
    OP = mybir.AluOpType

    nc = bacc.Bacc("TRN2", target_bir_lowering=False, debug=False,
                   num_devices=NCORES)

    def din(name, shape):
        return nc.dram_tensor(name, list(shape), f32, kind="ExternalInput")

    # ---- external inputs (per device) ----
    i_at = din("at", [NPAD, RPD])          # A[rows_d,:].T  (n, m_local)
    i_mfT = din("mfT", [768, RPD])
    i_ufT = din("ufT", [512, RPD])
    i_bT = din("bT", [P, RPD])             # base_w^T
    i_mi = din("mi", [P, RPD])             # item mask (replicated rows)
    i_mu = din("mu", [P, RPD])
    i_wme1 = din("wme1", [768, P])
    i_wme2 = din("wme2", [P, P])
    i_wue1 = din("wue1", [512, P])
    i_wue2 = din("wue2", [P, P])
    i_wcmv = din("wcmv", [P, P])
    i_wcmo = din("wcmo", [P, P])
    i_wmg = din("wmg", [256, P])
    i_wug = din("wug", [256, P])
    # bias columns: 0 me_b1, 1 me_b2, 2 ue_b1, 3 ue_b2, 4 cm_vb, 5 2*cm_ob,
    # 6 mg_b, 7 ug_b, 8 qb0', 9 qb1', 10 kb0, 11 kb1
    i_bc = din("bc", [P, 12])
    i_wq = din("wq", [L, P, P])            # qw / sqrt(D)
    i_wk = din("wk", [L, P, P])            # rel_mean[:,None] * kw
    i_wv = din("wv", [L, P, P])
    i_vbb = din("vbb", [P, L * P])         # vb broadcast tiles
    i_wla = din("wla", [384, P])
    i_lab = din("lab", [1, P])

    o_fin = nc.dram_tensor("o_fin", [RPD, P], f32, kind="ExternalOutput")
    o_mmT = nc.dram_tensor("o_mmT", [P, RPD], f32, kind="ExternalOutput")

    # ---- internal DRAM ----
    e_locs = [nc.dram_tensor(f"e_loc{l}", [RPD, P], f32) for l in range(L)]
    e_full = [nc.dram_tensor(f"e_full{l}", [NPAD, P], f32, addr_space="Shared")
              for l in range(L)]
    sT_locs = [nc.dram_tensor(f"sT_loc{l}", [P, RPD], f32) for l in range(L)]
    sT_fulls = [nc.dram_tensor(f"sT_full{l}", [NCORES, P, RPD], f32,
                               addr_space="Shared") for l in range(L)]

    ident_np = np.eye(P, dtype=np.float32)
    c_ident = nc.inline_tensor(ident_np, name="c_ident")
    c_ones = nc.inline_tensor(np.ones((1, P), np.float32), name="c_ones")

    RG = [list(range(NCORES))]

    with TileContext(nc) as tc:
        with (
            tc.tile_pool(name="wts", bufs=1) as wts,
            tc.tile_pool(name="big", bufs=1) as bigp,
            tc.tile_pool(name="sc2", bufs=1) as sc2,
            tc.tile_pool(name="str", bufs=3) as strm,
            tc.tile_pool(name="sm", bufs=3) as smp,
            tc.tile_pool(name="ps", bufs=2, space="PSUM") as ps,
            tc.tile_pool(name="psT", bufs=2, space="PSUM") as psT,
            tc.tile_pool(name="psacc", bufs=2, space="PSUM") as psacc,
        ):
            # ---------- persistent SBUF ----------
            ident = wts.tile([P, P], f32)
            nc.sync.dma_start(ident[:], c_ident.ap())
            ones1 = wts.tile([1, P], f32)
            nc.sync.dma_start(ones1[:], c_ones.ap())
            bc = wts.tile([P, 12], f32)
            nc.sync.dma_start(bc[:], i_bc.ap())
            vbb = wts.tile([P, L * P], f32)
            nc.sync.dma_start(vbb[:], i_vbb.ap())
            wq = [wts.tile([P, P], f32, tag=f"wq{l}", name=f"wq{l}") for l in range(L)]
            wk = [wts.tile([P, P], f32, tag=f"wk{l}", name=f"wk{l}") for l in range(L)]
            wv = [wts.tile([P, P], f32, tag=f"wv{l}", name=f"wv{l}") for l in range(L)]
            for l in range(L):
                nc.sync.dma_start(wq[l][:], i_wq.ap()[l])
                nc.sync.dma_start(wk[l][:], i_wk.ap()[l])
                nc.sync.dma_start(wv[l][:], i_wv.ap()[l])

            eT = [bigp.tile([P, RPD], f32, tag=f"eT{l}", name=f"eT{l}") for l in range(L + 1)]
            qnT = bigp.tile([P, RPD], f32)
            knT = bigp.tile([P, NPAD], f32)
            vn_sb = bigp.tile([P, NKT, P], f32)      # 63 tiles [128n,128d]
            u_scr = bigp.tile([P, CH], f32)
            den16 = bigp.tile([P, NCH], f32)

            HC = RPD // 512  # 2 chunks of 512

            def mm_chain(out_sb, lhsT_tiles, rhs_sb, bias_col, act, scale=1.0):
                """out_sb[:, h*512:] = act(sum_k lhsT_k.T @ rhs_k + bias)."""
                for h in range(HC):
                    pst = ps.tile([P, 512], f32, tag="gen")
                    nk = len(lhsT_tiles)
                    for k, (lt, rt) in enumerate(zip(lhsT_tiles, rhs_sb)):
                        nc.tensor.matmul(pst[:], lt, rt[:, h * 512:(h + 1) * 512],
                                         start=(k == 0), stop=(k == nk - 1))
                    nc.scalar.activation(out_sb[:, h * 512:(h + 1) * 512], pst[:],
                                         act, bias=bias_col, scale=scale)

            # ---------- prologue ----------
            with (
                tc.tile_pool(name="proW", bufs=1) as proW,
                tc.tile_pool(name="proB", bufs=8) as proB,
            ):
                def chain_dram(out_sb, lhsT_tiles, rhs_dram, bias_col, act,
                               scale=1.0):
                    nk = len(lhsT_tiles)
                    for h in range(HC):
                        pst = ps.tile([P, 512], f32, tag="gen")
                        for k in range(nk):
                            rt = proB.tile([P, 512], f32, tag="pstr",
                                           name=f"pstr_{h}_{k}")
                            nc.sync.dma_start(
                                rt[:],
                                rhs_dram.ap()[k * P:(k + 1) * P,
                                              h * 512:(h + 1) * 512])
                            nc.tensor.matmul(pst[:], lhsT_tiles[k], rt[:],
                                             start=(k == 0), stop=(k == nk - 1))
                        nc.scalar.activation(out_sb[:, h * 512:(h + 1) * 512],
                                             pst[:], act, bias=bias_col,
                                             scale=scale)

                wme1 = [proW.tile([P, P], f32, tag=f"wme1_{k}",
                                  name=f"wme1_{k}") for k in range(6)]
                for k in range(6):
                    nc.sync.dma_start(wme1[k][:],
                                      i_wme1.ap()[k * P:(k + 1) * P, :])
                wme2 = proW.tile([P, P], f32)
                nc.sync.dma_start(wme2[:], i_wme2.ap())
                wue1 = [proW.tile([P, P], f32, tag=f"wue1_{k}",
                                  name=f"wue1_{k}") for k in range(4)]
                for k in range(4):
                    nc.sync.dma_start(wue1[k][:],
                                      i_wue1.ap()[k * P:(k + 1) * P, :])
                wue2 = proW.tile([P, P], f32)
                nc.sync.dma_start(wue2[:], i_wue2.ap())
                wcmv = proW.tile([P, P], f32)
                nc.sync.dma_start(wcmv[:], i_wcmv.ap())
                wcmo = proW.tile([P, P], f32)
                nc.sync.dma_start(wcmo[:], i_wcmo.ap())
                wmg = [proW.tile([P, P], f32, tag=f"wmg{k}", name=f"wmg{k}")
                       for k in range(2)]
                wug = [proW.tile([P, P], f32, tag=f"wug{k}", name=f"wug{k}")
                       for k in range(2)]
                for k in range(2):
                    nc.sync.dma_start(wmg[k][:],
                                      i_wmg.ap()[k * P:(k + 1) * P, :])
                    nc.sync.dma_start(wug[k][:],
                                      i_wug.ap()[k * P:(k + 1) * P, :])

                h1 = proB.tile([P, RPD], f32, tag="pbig")
                chain_dram(h1, [w[:] for w in wme1], i_mfT, bc[:, 0:1], AF.Relu)
                mmIT = proB.tile([P, RPD], f32, tag="pbig")
                mm_chain(mmIT, [wme2[:]], [h1[:]], bc[:, 1:2], AF.Relu)
                vT = proB.tile([P, RPD], f32, tag="pbig")
                mm_chain(vT, [wcmv[:]], [mmIT[:]], bc[:, 4:5], AF.Identity)
                mmI2T = proB.tile([P, RPD], f32, tag="pbig")
                mm_chain(mmI2T, [wcmo[:]], [vT[:]], bc[:, 5:6], AF.Identity,
                         scale=2.0)
                nc.sync.dma_start(o_mmT.ap(), mmI2T[:])

                h1u = proB.tile([P, RPD], f32, tag="pbig")
                chain_dram(h1u, [w[:] for w in wue1], i_ufT, bc[:, 2:3],
                           AF.Relu)
                mmUT = proB.tile([P, RPD], f32, tag="pbig")
                mm_chain(mmUT, [wue2[:]], [h1u[:]], bc[:, 3:4], AF.Relu)

                bT = proB.tile([P, RPD], f32, tag="pbig")
                nc.sync.dma_start(bT[:], i_bT.ap())

                # item path
                s1 = proB.tile([P, RPD], f32, tag="pbig")
                mm_chain(s1, [wmg[0][:], wmg[1][:]], [bT[:], mmI2T[:]],
                         bc[:, 6:7], AF.Sigmoid)
                mgT = proB.tile([P, RPD], f32, tag="pbig")
                for h in range(HC):
                    sl = slice(h * 512, (h + 1) * 512)
                    nc.scalar.activation(mgT[:, sl], s1[:, sl], AF.Sigmoid,
                                         scale=2.0)
                mi = proB.tile([P, RPD], f32, tag="pbig")
                nc.sync.dma_start(mi[:], i_mi.ap())
                tA = proB.tile([P, RPD], f32, tag="pbig")
                nc.vector.tensor_sub(tA[:], bT[:], mmI2T[:])
                nc.vector.tensor_mul(tA[:], mgT[:], tA[:])
                nc.vector.tensor_add(tA[:], tA[:], mmI2T[:])
                nc.vector.scalar_tensor_tensor(
                    out=tA[:], in0=bT[:], scalar=-0.9, in1=tA[:],
                    op0=OP.mult, op1=OP.add)
                nc.vector.tensor_mul(tA[:], mi[:], tA[:])

                # user path
                s1b = proB.tile([P, RPD], f32, tag="pbig")
                mm_chain(s1b, [wug[0][:], wug[1][:]], [bT[:], mmUT[:]],
                         bc[:, 7:8], AF.Sigmoid)
                ugT = proB.tile([P, RPD], f32, tag="pbig")
                for h in range(HC):
                    sl = slice(h * 512, (h + 1) * 512)
                    nc.scalar.activation(ugT[:, sl], s1b[:, sl], AF.Sigmoid,
                                         scale=2.0)
                mu = proB.tile([P, RPD], f32, tag="pbig")
                nc.sync.dma_start(mu[:], i_mu.ap())
                tB = proB.tile([P, RPD], f32, tag="pbig")
                nc.vector.tensor_sub(tB[:], bT[:], mmUT[:])
                nc.vector.tensor_mul(tB[:], ugT[:], tB[:])
                nc.vector.tensor_add(tB[:], tB[:], mmUT[:])
                nc.vector.scalar_tensor_tensor(
                    out=tB[:], in0=bT[:], scalar=-0.9, in1=tB[:],
                    op0=OP.mult, op1=OP.add)
                nc.vector.tensor_mul(tB[:], mu[:], tB[:])

                e0T = eT[0]
                nc.vector.tensor_add(tA[:], tA[:], bT[:])
                nc.vector.tensor_add(e0T[:], tA[:], tB[:])

                for mt in range(MT):
                    pt = psT.tile([P, P], f32, tag="tp")
                    nc.tensor.transpose(pt[:], e0T[:, mt * P:(mt + 1) * P],
                                        ident[:])
                    e0n = smp.tile([P, P], f32, tag="blk")
                    nc.scalar.activation(e0n[:], pt[:], AF.Copy)
                    nc.sync.dma_start(e_locs[0].ap()[mt * P:(mt + 1) * P, :],
                                      e0n[:])
                nc.gpsimd.collective_compute(
                    "AllGather", OP.bypass, replica_groups=RG,
                    ins=[e_locs[0][:]], outs=[e_full[0][:]])

            # ---------- GNN layers ----------
            for l in range(L):
                # A) side^T = (A_dev @ e)^T : accumulate over 64 n-tiles
                psA = [psacc.tile([P, 512], f32, tag="acc", name=f"psA{h}")
                       for h in range(HC)]
                for k in range(AKT):
                    eL = strm.tile([P, P], f32, tag="eL")
                    nc.sync.dma_start(eL[:],
                                      e_full[l].ap()[k * P:(k + 1) * P, :])
                    for h in range(HC):
                        aTc = strm.tile([P, 512], f32, tag="aT")
                        nc.sync.dma_start(
                            aTc[:],
                            i_at.ap()[k * P:(k + 1) * P, h * 512:(h + 1) * 512])
                        nc.tensor.matmul(psA[h][:], eL[:], aTc[:],
                                         start=(k == 0), stop=(k == AKT - 1))
                for h in range(HC):
                    sA = strm.tile([P, 512], f32, tag="sA")
                    nc.scalar.activation(sA[:], psA[h][:], AF.Copy)
                    nc.sync.dma_start(sT_locs[l].ap()[:, h * 512:(h + 1) * 512],
                                      sA[:])
                nc.gpsimd.collective_compute(
                    "AllGather", OP.bypass, replica_groups=RG,
                    ins=[sT_locs[l][:]], outs=[sT_fulls[l][:]])

                # B) qn^T = wq.T @ e^T + qb
                mm_chain(qnT, [wq[l][:]], [eT[l][:]], bc[:, 8 + l:9 + l],
                         AF.Identity)

                # C) kn^T chunks + vn tiles from gathered side^T
                for c in range(NCH):
                    r, hh = c // 2, c % 2
                    sch = smp.tile([P, 512], f32, tag="sch")
                    nc.sync.dma_start(
                        sch[:], sT_fulls[l].ap()[r][:, hh * 512:(hh + 1) * 512])
                    psK = ps.tile([P, 512], f32, tag="gen")
                    nc.tensor.matmul(psK[:], wk[l][:], sch[:],
                                     start=True, stop=True)
                    nc.scalar.activation(knT[:, c * 512:(c + 1) * 512],
                                         psK[:], AF.Identity,
                                         bias=bc[:, 10 + l:11 + l])
                    for t4 in range(4):
                        nt = 4 * c + t4
                        if nt >= NKT:
                            continue
                        psV = ps.tile([P, P], f32, tag="gen")
                        nc.tensor.matmul(psV[:],
                                         sch[:, t4 * P:(t4 + 1) * P], wv[l][:],
                                         start=True, stop=True)
                        nc.scalar.activation(vn_sb[:, nt, :], psV[:], AF.Copy)

                # D) scores + topk-masked dense aggregation, per m-tile
                sc_chunks = NCH  # 16 chunks of 500
                for mt in range(MT):
                    sc = sc2.tile([P, N], f32, tag="sc")
                    cand = smp.tile([P, P], f32, tag="cand")
                    for c in range(sc_chunks):
                        psS = ps.tile([P, CH], f32, tag="psS")
                        nc.tensor.matmul(
                            psS[:], qnT[:, mt * P:(mt + 1) * P],
                            knT[:, c * CH:(c + 1) * CH],
                            start=True, stop=True)
                        nc.scalar.activation(sc[:, c * CH:(c + 1) * CH],
                                             psS[:], AF.Copy)
                        nc.vector.max(cand[:, c * 8:c * 8 + 8],
                                      sc[:, c * CH:(c + 1) * CH])
                    t8a = smp.tile([P, 8], f32, tag="t8a")
                    nc.vector.max(t8a[:], cand[:])
                    candr = smp.tile([P, P], f32, tag="candr")
                    nc.vector.match_replace(candr[:], t8a[:], cand[:], -BIG)
                    t8b = smp.tile([P, 8], f32, tag="t8b")
                    nc.vector.max(t8b[:], candr[:])
                    nrm = smp.tile([P, 1], f32, tag="nrm")
                    nc.vector.tensor_scalar(out=nrm[:], in0=t8a[:, 0:1],
                                            scalar1=-1.0, scalar2=None,
                                            op0=OP.mult)
                    # per chunk: u=(s-t16)*BIG; u=min(u,0)+s; s=exp(u-rowmax)
                    for c in range(NCH):
                        cs = slice(c * CH, (c + 1) * CH)
                        nc.vector.tensor_scalar(
                            out=u_scr[:], in0=sc[:, cs], scalar1=t8b[:, 7:8],
                            scalar2=BIG, op0=OP.subtract, op1=OP.mult)
                        nc.vector.scalar_tensor_tensor(
                            out=u_scr[:], in0=u_scr[:], scalar=0.0,
                            in1=sc[:, cs], op0=OP.min, op1=OP.add)
                        nc.scalar.activation(sc[:, cs], u_scr[:], AF.Exp,
                                             bias=nrm[:],
                                             accum_out=den16[:, c:c + 1])
                    den = smp.tile([P, 1], f32, tag="den")
                    nc.vector.tensor_reduce(out=den[:], in_=den16[:],
                                            axis=mybir.AxisListType.X,
                                            op=OP.add)
                    rden = smp.tile([P, 1], f32, tag="rden")
                    nc.vector.reciprocal(rden[:], den[:])
                    # agg = (w @ vn) * rden + vb ; relu
                    psG = psacc.tile([P, 512], f32, tag="acc")
                    for k in range(NKT):
                        kw_ = P if k < NKT - 1 else N - (NKT - 1) * P
                        pt = psT.tile([P, P], f32, tag="tp")
                        nc.tensor.transpose(pt[:kw_, :],
                                            sc[:, k * P:k * P + kw_], ident[:])
                        wTt = strm.tile([P, P], f32, tag="wTt")
                        nc.scalar.activation(wTt[:kw_, :], pt[:kw_, :], AF.Copy)
                        nc.tensor.matmul(psG[:, 0:P], wTt[:kw_, :],
                                         vn_sb[:kw_, k, :],
                                         start=(k == 0), stop=(k == NKT - 1))
                    agg = smp.tile([P, P], f32, tag="blk")
                    nc.scalar.activation(agg[:], psG[:, 0:P], AF.Copy,
                                         scale=rden[:])
                    nc.vector.tensor_add(agg[:], agg[:],
                                         vbb[:, l * P:(l + 1) * P])
                    enx = smp.tile([P, P], f32, tag="blk")
                    nc.scalar.activation(enx[:], agg[:], AF.Relu)
                    if l == 0:
                        nc.sync.dma_start(
                            e_locs[1].ap()[mt * P:(mt + 1) * P, :], enx[:])
                    # e_next^T tile
                    pte = psT.tile([P, P], f32, tag="tp")
                    nc.tensor.transpose(pte[:], enx[:], ident[:])
                    nc.scalar.activation(eT[l + 1][:, mt * P:(mt + 1) * P],
                                         pte[:], AF.Copy)
                if l == 0:
                    nc.gpsimd.collective_compute(
                        "AllGather", OP.bypass, replica_groups=RG,
                        ins=[e_locs[1][:]], outs=[e_full[1][:]])

            # ---------- epilogue: final = cat(e0,e1,e2)@la_w + la_b; norm ----
            wla = [wts.tile([P, P], f32, tag=f"wla{k}", name=f"wla{k}") for k in range(3)]
            for k in range(3):
                nc.sync.dma_start(wla[k][:], i_wla.ap()[k * P:(k + 1) * P, :])
            lab = wts.tile([1, P], f32)
            nc.sync.dma_start(lab[:], i_lab.ap())
            for mt in range(MT):
                psF = ps.tile([P, P], f32, tag="gen")
                for k in range(3):
                    nc.tensor.matmul(psF[:], eT[k][:, mt * P:(mt + 1) * P],
                                     wla[k][:], start=(k == 0), stop=False)
                nc.tensor.matmul(psF[:], ones1[:], lab[:],
                                 start=False, stop=True)
                zsb = smp.tile([P, P], f32, tag="blk")
                nc.scalar.activation(zsb[:], psF[:], AF.Copy)
                junk = smp.tile([P, P], f32, tag="blk")
                ssq = smp.tile([P, 1], f32, tag="ssq")
                nc.vector.scalar_tensor_tensor(
                    out=junk[:], in0=zsb[:], scalar=0.0, in1=zsb[:],
                    op0=OP.add, op1=OP.mult, accum_out=ssq[:])
                nrm2 = smp.tile([P, 1], f32, tag="nrm2")
                nc.scalar.activation(nrm2[:], ssq[:], AF.Sqrt)
                nc.vector.tensor_scalar_max(nrm2[:], nrm2[:], 1e-12)
                rn = smp.tile([P, 1], f32, tag="rn")
                nc.vector.reciprocal(rn[:], nrm2[:])
                fin = smp.tile([P, P], f32, tag="blk")
                nc.scalar.activation(fin[:], zsb[:], AF.Copy, scale=rn[:])
                nc.sync.dma_start(o_fin.ap()[mt * P:(mt + 1) * P, :], fin[:])

    nc.compile()
    return nc


def _host_prep(inputs):
    """Build per-device input maps."""
    f = np.float32
    uw = inputs["user_w"].astype(f)
    iw = inputs["item_w"].astype(f)
    ew = inputs["ent_w"].astype(f)
    base = np.zeros((NPAD, D), f)
    base[0:N_USERS] = uw
    base[N_USERS:N_USERS + N_ITEMS] = iw
    base[N_USERS + N_ITEMS:N] = ew
    mask_u = np.zeros(NPAD, f)
    mask_u[0:N_USERS] = 1.0
    mask_i = np.zeros(NPAD, f)
    mask_i[N_USERS:N_USERS + N_ITEMS] = 1.0

    mfTp = np.zeros((768, NPAD), f)
    mfTp[:, N_USERS:N_USERS + N_ITEMS] = inputs["mfeat"].astype(f).T
    ufTp = np.zeros((512, NPAD), f)
    ufTp[:, 0:N_USERS] = inputs["ufeat"].astype(f).T

    A = np.zeros((NPAD, NPAD), f)
    np.add.at(A, (inputs["adj_row"].astype(np.int64),
                  inputs["adj_col"].astype(np.int64)),
              inputs["adj_data"].astype(f))

    rel_mean = inputs["rel_w"].astype(f).mean(axis=0)
    sq = f(1.0) / np.sqrt(f(D))
    wq = (inputs["gnn_qw"].astype(f) * sq)
    wk = (rel_mean[None, :, None] * inputs["gnn_kw"].astype(f))
    wv = inputs["gnn_vw"].astype(f)
    qb = inputs["gnn_qb"].astype(f) * sq
    kb = inputs["gnn_kb"].astype(f)
    vb = inputs["gnn_vb"].astype(f)

    bc = np.zeros((P, 12), f)
    bc[:, 0] = inputs["me_b1"]
    bc[:, 1] = inputs["me_b2"]
    bc[:, 2] = inputs["ue_b1"]
    bc[:, 3] = inputs["ue_b2"]
    bc[:, 4] = inputs["cm_vb"]
    bc[:, 5] = 2.0 * inputs["cm_ob"].astype(f)
    bc[:, 6] = inputs["mg_b"]
    bc[:, 7] = inputs["ug_b"]
    bc[:, 8] = qb[0]
    bc[:, 9] = qb[1]
    bc[:, 10] = kb[0]
    bc[:, 11] = kb[1]

    vbb = np.zeros((P, L * P), f)
    for l in range(L):
        vbb[:, l * P:(l + 1) * P] = np.tile(vb[l][None, :], (P, 1))

    in_maps = []
    for dv in range(NCORES):
        rows = slice(dv * RPD, (dv + 1) * RPD)
        in_maps.append(dict(
            at=np.ascontiguousarray(A[rows, :].T),
            mfT=np.ascontiguousarray(mfTp[:, rows]),
            ufT=np.ascontiguousarray(ufTp[:, rows]),
            bT=np.ascontiguousarray(base[rows, :].T),
            mi=np.tile(mask_i[rows][None, :], (P, 1)),
            mu=np.tile(mask_u[rows][None, :], (P, 1)),
            wme1=inputs["me_w1"].astype(f), wme2=inputs["me_w2"].astype(f),
            wue1=inputs["ue_w1"].astype(f), wue2=inputs["ue_w2"].astype(f),
            wcmv=inputs["cm_vw"].astype(f), wcmo=inputs["cm_ow"].astype(f),
            wmg=inputs["mg_w"].astype(f), wug=inputs["ug_w"].astype(f),
            bc=bc, wq=wq, wk=wk, wv=wv, vbb=vbb,
            wla=inputs["la_w"].astype(f),
            lab=inputs["la_b"].astype(f)[None, :],
        ))
    return in_maps


def kernel(**inputs):
    from concourse.bass_utils import run_bass_kernel_spmd

    if "nc" not in _PROGRAM_CACHE:
        _PROGRAM_CACHE["nc"] = _build_program()
    nc = _PROGRAM_CACHE["nc"]

    in_maps = _host_prep(inputs)
    _PROGRAM_CACHE["last_in_maps"] = in_maps
    r = run_bass_kernel_spmd(nc, in_maps, list(range(NCORES)))
    _PROGRAM_CACHE["last_result"] = r
    res = r.results

    fin = np.concatenate([res[dv]["o_fin"] for dv in range(NCORES)], axis=0)
    mmT = np.concatenate([res[dv]["o_mmT"] for dv in range(NCORES)], axis=1)
    user_out = fin[0:N_USERS]
    item_out = fin[N_USERS:N_USERS + N_ITEMS]
    mm_item = np.ascontiguousarray(mmT[:, N_USERS:N_USERS + N_ITEMS].T)
    item_w = np.asarray(inputs["item_w"], np.float32).copy()
    return (user_out, item_out, item_w, mm_item)


# revision 19
# speedup vs baseline: 1.0341x; 1.0341x over previous
"""Trainium2 Bass kernel for nn_GCMKGATCL_Ablation (GNN message passing).

Self-contained: takes FULL unsharded inputs, shards rows across 8 NeuronCores,
runs a Bass/Tile SPMD kernel (fp32 everywhere on the selection path), returns
the full outputs matching reference().

Sharding: node dim N=8000 padded to 8192, 1024 rows per core. Dense adjacency
A^T slices per core; per-layer AllGather of side^T and of e. Top-16 per row via
chunked max8 + threshold-masked dense softmax-matmul aggregation (no dynamic
gather — indirect DMA unsupported in this environment).
"""

import numpy as np

NCORES = 8
P = 128
D = 128
N_USERS, N_ITEMS, N_ENT = 2500, 3000, 2500
N = N_USERS + N_ITEMS + N_ENT          # 8000
NPAD = 8192
RPD = NPAD // NCORES                   # 1024 rows per device
MT = RPD // P                          # 8 m-tiles per device
NCH = 16                               # score chunks
CH = N // NCH                          # 500 cols per chunk
NKT = (N + P - 1) // P                 # 63 agg contraction tiles (62*128+64)
AKT = NPAD // P                        # 64 side contraction tiles
L = 2
GATE_T = 0.5
BIG = 1.0e30

_PROGRAM_CACHE = {}


def _build_program():
    import concourse.bacc as bacc
    import concourse.bass as bass
    import concourse.mybir as mybir
    from concourse.tile import TileContext

    f32 = mybir.dt.float32
    AF = mybir.ActivationFunctionType
    OP = mybir.AluOpType

    nc = bacc.Bacc("TRN2", target_bir_lowering=False, debug=False,
                   num_devices=NCORES)

    def din(name, shape):
        return nc.dram_tensor(name, list(shape), f32, kind="ExternalInput")

    # ---- external inputs (per device) ----
    i_at = din("at", [NPAD, RPD])          # A[rows_d,:].T  (n, m_local)
    i_mfT = din("mfT", [768, RPD])
    i_ufT = din("ufT", [512, RPD])
    i_bT = din("bT", [P, RPD])             # base_w^T
    i_mi = din("mi", [P, RPD])             # item mask (replicated rows)
    i_mu = din("mu", [P, RPD])
    i_wme1 = din("wme1", [768, P])
    i_wme2 = din("wme2", [P, P])
    i_wue1 = din("wue1", [512, P])
    i_wue2 = din("wue2", [P, P])
    i_wcmv = din("wcmv", [P, P])
    i_wcmo = din("wcmo", [P, P])
    i_wmg = din("wmg", [256, P])
    i_wug = din("wug", [256, P])
    # bias columns: 0 me_b1, 1 me_b2, 2 ue_b1, 3 ue_b2, 4 cm_vb, 5 2*cm_ob,
    # 6 mg_b, 7 ug_b, 8 qb0', 9 qb1', 10 kb0, 11 kb1
    i_bc = din("bc", [P, 12])
    i_wq = din("wq", [L, P, P])            # qw / sqrt(D)
    i_wk = din("wk", [L, P, P])            # rel_mean[:,None] * kw
    i_wv = din("wv", [L, P, P])
    i_vbb = din("vbb", [P, L * P])         # vb broadcast tiles
    i_wla = din("wla", [384, P])
    i_lab = din("lab", [1, P])

    o_fin = nc.dram_tensor("o_fin", [RPD, P], f32, kind="ExternalOutput")
    o_mmT = nc.dram_tensor("o_mmT", [P, RPD], f32, kind="ExternalOutput")

    # ---- internal DRAM ----
    e_locs = [nc.dram_tensor(f"e_loc{l}", [RPD, P], f32) for l in range(L)]
    e_full = [nc.dram_tensor(f"e_full{l}", [NPAD, P], f32, addr_space="Shared")
              for l in range(L)]
    sT_locs = [nc.dram_tensor(f"sT_loc{l}", [P, RPD], f32) for l in range(L)]
    sT_fulls = [nc.dram_tensor(f"sT_full{l}", [NCORES, P, RPD], f32,
                               addr_space="Shared") for l in range(L)]

    ident_np = np.eye(P, dtype=np.float32)
    c_ident = nc.inline_tensor(ident_np, name="c_ident")
    c_ones = nc.inline_tensor(np.ones((1, P), np.float32), name="c_ones")

    RG = [list(range(NCORES))]

    with TileContext(nc) as tc:
        with (
            tc.tile_pool(name="wts", bufs=1) as wts,
            tc.tile_pool(name="big", bufs=1) as bigp,
            tc.tile_pool(name="sc2", bufs=1) as sc2,
            tc.tile_pool(name="str", bufs=3) as strm,
            tc.tile_pool(name="sm", bufs=3) as smp,
            tc.tile_pool(name="ps", bufs=2, space="PSUM") as ps,
            tc.tile_pool(name="psT", bufs=2, space="PSUM") as psT,
            tc.tile_pool(name="psacc", bufs=2, space="PSUM") as psacc,
        ):
            # ---------- persistent SBUF ----------
            ident = wts.tile([P, P], f32)
            nc.sync.dma_start(ident[:], c_ident.ap())
            ones1 = wts.tile([1, P], f32)
            nc.sync.dma_start(ones1[:], c_ones.ap())
            bc = wts.tile([P, 12], f32)
            nc.sync.dma_start(bc[:], i_bc.ap())
            vbb = wts.tile([P, L * P], f32)
            nc.sync.dma_start(vbb[:], i_vbb.ap())
            wq = [wts.tile([P, P], f32, tag=f"wq{l}", name=f"wq{l}") for l in range(L)]
            wk = [wts.tile([P, P], f32, tag=f"wk{l}", name=f"wk{l}") for l in range(L)]
            wv = [wts.tile([P, P], f32, tag=f"wv{l}", name=f"wv{l}") for l in range(L)]
            for l in range(L):
                nc.sync.dma_start(wq[l][:], i_wq.ap()[l])
                nc.sync.dma_start(wk[l][:], i_wk.ap()[l])
                nc.sync.dma_start(wv[l][:], i_wv.ap()[l])

            eT = [bigp.tile([P, RPD], f32, tag=f"eT{l}", name=f"eT{l}") for l in range(L + 1)]
            qnT = bigp.tile([P, RPD], f32)
            knT = bigp.tile([P, NPAD], f32)
            vn_sb = bigp.tile([P, NKT, P], f32)      # 63 tiles [128n,128d]
            u_scr = bigp.tile([P, CH], f32)
            den16 = bigp.tile([P, NCH], f32)
            bf16 = mybir.dt.bfloat16
            scb16 = bigp.tile([P, NKT * P], bf16)

            nc.vector.memset(scb16[:, N:NKT * P], 0.0)

            HC = RPD // 512  # 2 chunks of 512

            def mm_chain(out_sb, lhsT_tiles, rhs_sb, bias_col, act, scale=1.0):
                """out_sb[:, h*512:] = act(sum_k lhsT_k.T @ rhs_k + bias)."""
                for h in range(HC):
                    pst = ps.tile([P, 512], f32, tag="gen")
                    nk = len(lhsT_tiles)
                    for k, (lt, rt) in enumerate(zip(lhsT_tiles, rhs_sb)):
                        nc.tensor.matmul(pst[:], lt, rt[:, h * 512:(h + 1) * 512],
                                         start=(k == 0), stop=(k == nk - 1))
                    nc.scalar.activation(out_sb[:, h * 512:(h + 1) * 512], pst[:],
                                         act, bias=bias_col, scale=scale)

            # ---------- prologue ----------
            with (
                tc.tile_pool(name="proW", bufs=1) as proW,
                tc.tile_pool(name="proB", bufs=7) as proB,
            ):
                def chain_dram(out_sb, lhsT_tiles, rhs_dram, bias_col, act,
                               scale=1.0):
                    nk = len(lhsT_tiles)
                    for h in range(HC):
                        pst = ps.tile([P, 512], f32, tag="gen")
                        for k in range(nk):
                            rt = strm.tile([P, 512], f32, tag="pstr",
                                           name=f"pstr_{h}_{k}")
                            nc.sync.dma_start(
                                rt[:],
                                rhs_dram.ap()[k * P:(k + 1) * P,
                                              h * 512:(h + 1) * 512])
                            nc.tensor.matmul(pst[:], lhsT_tiles[k], rt[:],
                                             start=(k == 0), stop=(k == nk - 1))
                        nc.scalar.activation(out_sb[:, h * 512:(h + 1) * 512],
                                             pst[:], act, bias=bias_col,
                                             scale=scale)

                wme1 = [proW.tile([P, P], f32, tag=f"wme1_{k}",
                                  name=f"wme1_{k}") for k in range(6)]
                for k in range(6):
                    nc.sync.dma_start(wme1[k][:],
                                      i_wme1.ap()[k * P:(k + 1) * P, :])
                wme2 = proW.tile([P, P], f32)
                nc.sync.dma_start(wme2[:], i_wme2.ap())
                wue1 = [proW.tile([P, P], f32, tag=f"wue1_{k}",
                                  name=f"wue1_{k}") for k in range(4)]
                for k in range(4):
                    nc.sync.dma_start(wue1[k][:],
                                      i_wue1.ap()[k * P:(k + 1) * P, :])
                wue2 = proW.tile([P, P], f32)
                nc.sync.dma_start(wue2[:], i_wue2.ap())
                wcmv = proW.tile([P, P], f32)
                nc.sync.dma_start(wcmv[:], i_wcmv.ap())
                wcmo = proW.tile([P, P], f32)
                nc.sync.dma_start(wcmo[:], i_wcmo.ap())
                wmg = [proW.tile([P, P], f32, tag=f"wmg{k}", name=f"wmg{k}")
                       for k in range(2)]
                wug = [proW.tile([P, P], f32, tag=f"wug{k}", name=f"wug{k}")
                       for k in range(2)]
                for k in range(2):
                    nc.sync.dma_start(wmg[k][:],
                                      i_wmg.ap()[k * P:(k + 1) * P, :])
                    nc.sync.dma_start(wug[k][:],
                                      i_wug.ap()[k * P:(k + 1) * P, :])

                h1 = proB.tile([P, RPD], f32, tag="pbig")
                chain_dram(h1, [w[:] for w in wme1], i_mfT, bc[:, 0:1], AF.Relu)
                mmIT = proB.tile([P, RPD], f32, tag="pbig")
                mm_chain(mmIT, [wme2[:]], [h1[:]], bc[:, 1:2], AF.Relu)
                vT = proB.tile([P, RPD], f32, tag="pbig")
                mm_chain(vT, [wcmv[:]], [mmIT[:]], bc[:, 4:5], AF.Identity)
                mmI2T = proB.tile([P, RPD], f32, tag="pbig")
                mm_chain(mmI2T, [wcmo[:]], [vT[:]], bc[:, 5:6], AF.Identity,
                         scale=2.0)
                nc.sync.dma_start(o_mmT.ap(), mmI2T[:])

                h1u = proB.tile([P, RPD], f32, tag="pbig")
                chain_dram(h1u, [w[:] for w in wue1], i_ufT, bc[:, 2:3],
                           AF.Relu)
                mmUT = proB.tile([P, RPD], f32, tag="pbig")
                mm_chain(mmUT, [wue2[:]], [h1u[:]], bc[:, 3:4], AF.Relu)

                bT = proB.tile([P, RPD], f32, tag="pbig")
                nc.sync.dma_start(bT[:], i_bT.ap())

                # item path
                s1 = proB.tile([P, RPD], f32, tag="pbig")
                mm_chain(s1, [wmg[0][:], wmg[1][:]], [bT[:], mmI2T[:]],
                         bc[:, 6:7], AF.Sigmoid)
                mgT = proB.tile([P, RPD], f32, tag="pbig")
                for h in range(HC):
                    sl = slice(h * 512, (h + 1) * 512)
                    nc.scalar.activation(mgT[:, sl], s1[:, sl], AF.Sigmoid,
                                         scale=2.0)
                mi = proB.tile([P, RPD], f32, tag="pbig")
                nc.sync.dma_start(mi[:], i_mi.ap())
                tA = proB.tile([P, RPD], f32, tag="pbig")
                nc.vector.tensor_sub(tA[:], bT[:], mmI2T[:])
                nc.vector.tensor_mul(tA[:], mgT[:], tA[:])
                nc.vector.tensor_add(tA[:], tA[:], mmI2T[:])
                nc.vector.scalar_tensor_tensor(
                    out=tA[:], in0=bT[:], scalar=-0.9, in1=tA[:],
                    op0=OP.mult, op1=OP.add)
                nc.vector.tensor_mul(tA[:], mi[:], tA[:])

                # user path
                s1b = proB.tile([P, RPD], f32, tag="pbig")
                mm_chain(s1b, [wug[0][:], wug[1][:]], [bT[:], mmUT[:]],
                         bc[:, 7:8], AF.Sigmoid)
                ugT = proB.tile([P, RPD], f32, tag="pbig")
                for h in range(HC):
                    sl = slice(h * 512, (h + 1) * 512)
                    nc.scalar.activation(ugT[:, sl], s1b[:, sl], AF.Sigmoid,
                                         scale=2.0)
                mu = proB.tile([P, RPD], f32, tag="pbig")
                nc.sync.dma_start(mu[:], i_mu.ap())
                tB = proB.tile([P, RPD], f32, tag="pbig")
                nc.vector.tensor_sub(tB[:], bT[:], mmUT[:])
                nc.vector.tensor_mul(tB[:], ugT[:], tB[:])
                nc.vector.tensor_add(tB[:], tB[:], mmUT[:])
                nc.vector.scalar_tensor_tensor(
                    out=tB[:], in0=bT[:], scalar=-0.9, in1=tB[:],
                    op0=OP.mult, op1=OP.add)
                nc.vector.tensor_mul(tB[:], mu[:], tB[:])

                e0T = eT[0]
                nc.vector.tensor_add(tA[:], tA[:], bT[:])
                nc.vector.tensor_add(e0T[:], tA[:], tB[:])

                for mt in range(MT):
                    pt = psT.tile([P, P], f32, tag="tp")
                    nc.tensor.transpose(pt[:], e0T[:, mt * P:(mt + 1) * P],
                                        ident[:])
                    e0n = smp.tile([P, P], f32, tag="blk")
                    nc.scalar.activation(e0n[:], pt[:], AF.Copy)
                    nc.sync.dma_start(e_locs[0].ap()[mt * P:(mt + 1) * P, :],
                                      e0n[:])
                nc.gpsimd.collective_compute(
                    "AllGather", OP.bypass, replica_groups=RG,
                    ins=[e_locs[0][:]], outs=[e_full[0][:]])

            # ---------- GNN layers ----------
            for l in range(L):
                # A) side^T = (A_dev @ e)^T : accumulate over 64 n-tiles
                psA = [psacc.tile([P, 512], f32, tag="acc", name=f"psA{h}")
                       for h in range(HC)]
                for k in range(AKT):
                    eL = strm.tile([P, P], f32, tag="eL")
                    nc.sync.dma_start(eL[:],
                                      e_full[l].ap()[k * P:(k + 1) * P, :])
                    for h in range(HC):
                        aTc = strm.tile([P, 512], f32, tag="aT")
                        nc.sync.dma_start(
                            aTc[:],
                            i_at.ap()[k * P:(k + 1) * P, h * 512:(h + 1) * 512])
                        nc.tensor.matmul(psA[h][:], eL[:], aTc[:],
                                         start=(k == 0), stop=(k == AKT - 1))
                for h in range(HC):
                    sA = strm.tile([P, 512], f32, tag="sA")
                    nc.scalar.activation(sA[:], psA[h][:], AF.Copy)
                    nc.sync.dma_start(sT_locs[l].ap()[:, h * 512:(h + 1) * 512],
                                      sA[:])
                nc.gpsimd.collective_compute(
                    "AllGather", OP.bypass, replica_groups=RG,
                    ins=[sT_locs[l][:]], outs=[sT_fulls[l][:]])

                # B) qn^T = wq.T @ e^T + qb
                mm_chain(qnT, [wq[l][:]], [eT[l][:]], bc[:, 8 + l:9 + l],
                         AF.Identity)

                # C) kn^T chunks + vn tiles from gathered side^T
                for c in range(NCH):
                    r, hh = c // 2, c % 2
                    sch = smp.tile([P, 512], f32, tag="sch")
                    nc.sync.dma_start(
                        sch[:], sT_fulls[l].ap()[r][:, hh * 512:(hh + 1) * 512])
                    psK = ps.tile([P, 512], f32, tag="gen")
                    nc.tensor.matmul(psK[:], wk[l][:], sch[:],
                                     start=True, stop=True)
                    nc.scalar.activation(knT[:, c * 512:(c + 1) * 512],
                                         psK[:], AF.Identity,
                                         bias=bc[:, 10 + l:11 + l])
                    for t4 in range(4):
                        nt = 4 * c + t4
                        if nt >= NKT:
                            continue
                        psV = ps.tile([P, P], f32, tag="gen")
                        nc.tensor.matmul(psV[:],
                                         sch[:, t4 * P:(t4 + 1) * P], wv[l][:],
                                         start=True, stop=True)
                        if l == 0:
                            nc.scalar.activation(vn_sb[:, nt, :], psV[:],
                                                 AF.Copy)
                        else:
                            vn16 = vn_sb[:].bitcast(bf16)
                            nc.scalar.activation(vn16[:, nt, 0:P], psV[:],
                                                 AF.Copy)

                # D) scores + topk-masked dense aggregation, per m-tile
                sc_chunks = NCH  # 16 chunks of 500
                for mt in range(MT):
                    sc = sc2.tile([P, N], f32, tag="sc")
                    cand = smp.tile([P, P], f32, tag="cand")
                    for c in range(sc_chunks):
                        psS = ps.tile([P, CH], f32, tag="psS")
                        nc.tensor.matmul(
                            psS[:], qnT[:, mt * P:(mt + 1) * P],
                            knT[:, c * CH:(c + 1) * CH],
                            start=True, stop=True)
                        nc.scalar.activation(sc[:, c * CH:(c + 1) * CH],
                                             psS[:], AF.Copy)
                        nc.vector.max(cand[:, c * 8:c * 8 + 8],
                                      sc[:, c * CH:(c + 1) * CH])
                    t8a = smp.tile([P, 8], f32, tag="t8a")
                    nc.vector.max(t8a[:], cand[:])
                    candr = smp.tile([P, P], f32, tag="candr")
                    nc.vector.match_replace(candr[:], t8a[:], cand[:], -BIG)
                    t8b = smp.tile([P, 8], f32, tag="t8b")
                    nc.vector.max(t8b[:], candr[:])
                    nrm = smp.tile([P, 1], f32, tag="nrm")
                    nc.vector.tensor_scalar(out=nrm[:], in0=t8a[:, 0:1],
                                            scalar1=-1.0, scalar2=None,
                                            op0=OP.mult)
                    # per chunk: u=(s-t16)*BIG; u=min(u,0)+s; s=exp(u-rowmax)
                    for c in range(NCH):
                        cs = slice(c * CH, (c + 1) * CH)
                        nc.vector.tensor_scalar(
                            out=u_scr[:], in0=sc[:, cs], scalar1=t8b[:, 7:8],
                            scalar2=BIG, op0=OP.subtract, op1=OP.mult)
                        nc.vector.scalar_tensor_tensor(
                            out=u_scr[:], in0=u_scr[:], scalar=0.0,
                            in1=sc[:, cs], op0=OP.min, op1=OP.add)
                        nc.scalar.activation(sc[:, cs], u_scr[:], AF.Exp,
                                             bias=nrm[:],
                                             accum_out=den16[:, c:c + 1])
                    den = smp.tile([P, 1], f32, tag="den")
                    nc.vector.tensor_reduce(out=den[:], in_=den16[:],
                                            axis=mybir.AxisListType.X,
                                            op=OP.add)
                    rden = smp.tile([P, 1], f32, tag="rden")
                    nc.vector.reciprocal(rden[:], den[:])
                    # agg = (w @ vn) * rden + vb ; relu
                    psG = psacc.tile([P, 512], f32, tag="acc")
                    if l == 0:
                        for k in range(NKT):
                            kw_ = P if k < NKT - 1 else N - (NKT - 1) * P
                            pt = psT.tile([P, P], f32, tag="tp")
                            nc.tensor.transpose(pt[:kw_, :],
                                                sc[:, k * P:k * P + kw_],
                                                ident[:])
                            wTt = strm.tile([P, P], f32, tag="wTt")
                            nc.scalar.activation(wTt[:kw_, :], pt[:kw_, :],
                                                 AF.Copy)
                            nc.tensor.matmul(psG[:, 0:P], wTt[:kw_, :],
                                             vn_sb[:kw_, k, :],
                                             start=(k == 0),
                                             stop=(k == NKT - 1))
                    else:
                        for c4 in range(4):
                            nc.scalar.activation(
                                scb16[:, c4 * 2000:(c4 + 1) * 2000],
                                sc[:, c4 * 2000:(c4 + 1) * 2000], AF.Copy)
                        vn16 = vn_sb[:].bitcast(bf16)
                        for k in range(NKT):
                            kw_ = P if k < NKT - 1 else N - (NKT - 1) * P
                            wT16 = strm.tile([P, P], bf16, tag="wT16")
                            nc.sync.dma_start_transpose(
                                wT16[:], scb16[:, k * P:(k + 1) * P])
                            nc.tensor.matmul(psG[:, 0:P], wT16[:kw_, :],
                                             vn16[:kw_, k, 0:P],
                                             start=(k == 0),
                                             stop=(k == NKT - 1))
                    agg = smp.tile([P, P], f32, tag="blk")
                    nc.scalar.activation(agg[:], psG[:, 0:P], AF.Copy,
                                         scale=rden[:])
                    nc.vector.tensor_add(agg[:], agg[:],
                                         vbb[:, l * P:(l + 1) * P])
                    enx = smp.tile([P, P], f32, tag="blk")
                    nc.scalar.activation(enx[:], agg[:], AF.Relu)
                    if l == 0:
                        nc.sync.dma_start(
                            e_locs[1].ap()[mt * P:(mt + 1) * P, :], enx[:])
                    # e_next^T tile
                    pte = psT.tile([P, P], f32, tag="tp")
                    nc.tensor.transpose(pte[:], enx[:], ident[:])
                    nc.scalar.activation(eT[l + 1][:, mt * P:(mt + 1) * P],
                                         pte[:], AF.Copy)
                if l == 0:
                    nc.gpsimd.collective_compute(
                        "AllGather", OP.bypass, replica_groups=RG,
                        ins=[e_locs[1][:]], outs=[e_full[1][:]])

            # ---------- epilogue: final = cat(e0,e1,e2)@la_w + la_b; norm ----
            wla = [wts.tile([P, P], f32, tag=f"wla{k}", name=f"wla{k}") for k in range(3)]
            for k in range(3):
                nc.sync.dma_start(wla[k][:], i_wla.ap()[k * P:(k + 1) * P, :])
            lab = wts.tile([1, P], f32)
            nc.sync.dma_start(lab[:], i_lab.ap())
            for mt in range(MT):
                psF = ps.tile([P, P], f32, tag="gen")
                for k in range(3):
                    nc.tensor.matmul(psF[:], eT[k][:, mt * P:(mt + 1) * P],
                                     wla[k][:], start=(k == 0), stop=False)
                nc.tensor.matmul(psF[:], ones1[:], lab[:],
                                 start=False, stop=True)
                zsb = smp.tile([P, P], f32, tag="blk")
                nc.scalar.activation(zsb[:], psF[:], AF.Copy)
                junk = smp.tile([P, P], f32, tag="blk")
                ssq = smp.tile([P, 1], f32, tag="ssq")
                nc.vector.scalar_tensor_tensor(
                    out=junk[:], in0=zsb[:], scalar=0.0, in1=zsb[:],
                    op0=OP.add, op1=OP.mult, accum_out=ssq[:])
                nrm2 = smp.tile([P, 1], f32, tag="nrm2")
                nc.scalar.activation(nrm2[:], ssq[:], AF.Sqrt)
                nc.vector.tensor_scalar_max(nrm2[:], nrm2[:], 1e-12)
                rn = smp.tile([P, 1], f32, tag="rn")
                nc.vector.reciprocal(rn[:], nrm2[:])
                fin = smp.tile([P, P], f32, tag="blk")
                nc.scalar.activation(fin[:], zsb[:], AF.Copy, scale=rn[:])
                nc.sync.dma_start(o_fin.ap()[mt * P:(mt + 1) * P, :], fin[:])

    nc.compile()
    return nc


def _host_prep(inputs):
    """Build per-device input maps."""
    f = np.float32
    uw = inputs["user_w"].astype(f)
    iw = inputs["item_w"].astype(f)
    ew = inputs["ent_w"].astype(f)
    base = np.zeros((NPAD, D), f)
    base[0:N_USERS] = uw
    base[N_USERS:N_USERS + N_ITEMS] = iw
    base[N_USERS + N_ITEMS:N] = ew
    mask_u = np.zeros(NPAD, f)
    mask_u[0:N_USERS] = 1.0
    mask_i = np.zeros(NPAD, f)
    mask_i[N_USERS:N_USERS + N_ITEMS] = 1.0

    mfTp = np.zeros((768, NPAD), f)
    mfTp[:, N_USERS:N_USERS + N_ITEMS] = inputs["mfeat"].astype(f).T
    ufTp = np.zeros((512, NPAD), f)
    ufTp[:, 0:N_USERS] = inputs["ufeat"].astype(f).T

    A = np.zeros((NPAD, NPAD), f)
    np.add.at(A, (inputs["adj_row"].astype(np.int64),
                  inputs["adj_col"].astype(np.int64)),
              inputs["adj_data"].astype(f))

    rel_mean = inputs["rel_w"].astype(f).mean(axis=0)
    sq = f(1.0) / np.sqrt(f(D))
    wq = (inputs["gnn_qw"].astype(f) * sq)
    wk = (rel_mean[None, :, None] * inputs["gnn_kw"].astype(f))
    wv = inputs["gnn_vw"].astype(f)
    qb = inputs["gnn_qb"].astype(f) * sq
    kb = inputs["gnn_kb"].astype(f)
    vb = inputs["gnn_vb"].astype(f)

    bc = np.zeros((P, 12), f)
    bc[:, 0] = inputs["me_b1"]
    bc[:, 1] = inputs["me_b2"]
    bc[:, 2] = inputs["ue_b1"]
    bc[:, 3] = inputs["ue_b2"]
    bc[:, 4] = inputs["cm_vb"]
    bc[:, 5] = 2.0 * inputs["cm_ob"].astype(f)
    bc[:, 6] = inputs["mg_b"]
    bc[:, 7] = inputs["ug_b"]
    bc[:, 8] = qb[0]
    bc[:, 9] = qb[1]
    bc[:, 10] = kb[0]
    bc[:, 11] = kb[1]

    vbb = np.zeros((P, L * P), f)
    for l in range(L):
        vbb[:, l * P:(l + 1) * P] = np.tile(vb[l][None, :], (P, 1))

    in_maps = []
    for dv in range(NCORES):
        rows = slice(dv * RPD, (dv + 1) * RPD)
        in_maps.append(dict(
            at=np.ascontiguousarray(A[rows, :].T),
            mfT=np.ascontiguousarray(mfTp[:, rows]),
            ufT=np.ascontiguousarray(ufTp[:, rows]),
            bT=np.ascontiguousarray(base[rows, :].T),
            mi=np.tile(mask_i[rows][None, :], (P, 1)),
            mu=np.tile(mask_u[rows][None, :], (P, 1)),
            wme1=inputs["me_w1"].astype(f), wme2=inputs["me_w2"].astype(f),
            wue1=inputs["ue_w1"].astype(f), wue2=inputs["ue_w2"].astype(f),
            wcmv=inputs["cm_vw"].astype(f), wcmo=inputs["cm_ow"].astype(f),
            wmg=inputs["mg_w"].astype(f), wug=inputs["ug_w"].astype(f),
            bc=bc, wq=wq, wk=wk, wv=wv, vbb=vbb,
            wla=inputs["la_w"].astype(f),
            lab=inputs["la_b"].astype(f)[None, :],
        ))
    return in_maps


def kernel(**inputs):
    from concourse.bass_utils import run_bass_kernel_spmd

    if "nc" not in _PROGRAM_CACHE:
        _PROGRAM_CACHE["nc"] = _build_program()
    nc = _PROGRAM_CACHE["nc"]

    in_maps = _host_prep(inputs)
    _PROGRAM_CACHE["last_in_maps"] = in_maps
    r = run_bass_kernel_spmd(nc, in_maps, list(range(NCORES)))
    _PROGRAM_CACHE["last_result"] = r
    res = r.results

    fin = np.concatenate([res[dv]["o_fin"] for dv in range(NCORES)], axis=0)
    mmT = np.concatenate([res[dv]["o_mmT"] for dv in range(NCORES)], axis=1)
    user_out = fin[0:N_USERS]
    item_out = fin[N_USERS:N_USERS + N_ITEMS]
    mm_item = np.ascontiguousarray(mmT[:, N_USERS:N_USERS + N_ITEMS].T)
    item_w = np.asarray(inputs["item_w"], np.float32).copy()
    return (user_out, item_out, item_w, mm_item)


# revision 20
# speedup vs baseline: 1.0482x; 1.0136x over previous
"""Trainium2 Bass kernel for nn_GCMKGATCL_Ablation (GNN message passing).

Self-contained: takes FULL unsharded inputs, shards rows across 8 NeuronCores,
runs a Bass/Tile SPMD kernel (fp32 everywhere on the selection path), returns
the full outputs matching reference().

Sharding: node dim N=8000 padded to 8192, 1024 rows per core. Dense adjacency
A^T slices per core; per-layer AllGather of side^T and of e. Top-16 per row via
chunked max8 + threshold-masked dense softmax-matmul aggregation (no dynamic
gather — indirect DMA unsupported in this environment).
"""

import numpy as np

NCORES = 8
P = 128
D = 128
N_USERS, N_ITEMS, N_ENT = 2500, 3000, 2500
N = N_USERS + N_ITEMS + N_ENT          # 8000
NPAD = 8192
RPD = NPAD // NCORES                   # 1024 rows per device
MT = RPD // P                          # 8 m-tiles per device
NCH = 16                               # score chunks
CH = N // NCH                          # 500 cols per chunk
NKT = (N + P - 1) // P                 # 63 agg contraction tiles (62*128+64)
AKT = NPAD // P                        # 64 side contraction tiles
L = 2
GATE_T = 0.5
BIG = 1.0e30

_PROGRAM_CACHE = {}


def _build_program():
    import concourse.bacc as bacc
    import concourse.bass as bass
    import concourse.mybir as mybir
    from concourse.tile import TileContext

    f32 = mybir.dt.float32
    AF = mybir.ActivationFunctionType
    OP = mybir.AluOpType

    nc = bacc.Bacc("TRN2", target_bir_lowering=False, debug=False,
                   num_devices=NCORES)

    def din(name, shape):
        return nc.dram_tensor(name, list(shape), f32, kind="ExternalInput")

    # ---- external inputs (per device) ----
    i_at = din("at", [NPAD, RPD])          # A[rows_d,:].T  (n, m_local)
    i_mfT = din("mfT", [768, RPD])
    i_ufT = din("ufT", [512, RPD])
    i_bT = din("bT", [P, RPD])             # base_w^T
    i_mi = din("mi", [P, RPD])             # item mask (replicated rows)
    i_mu = din("mu", [P, RPD])
    i_wme1 = din("wme1", [768, P])
    i_wme2 = din("wme2", [P, P])
    i_wue1 = din("wue1", [512, P])
    i_wue2 = din("wue2", [P, P])
    i_wcmv = din("wcmv", [P, P])
    i_wcmo = din("wcmo", [P, P])
    i_wmg = din("wmg", [256, P])
    i_wug = din("wug", [256, P])
    # bias columns: 0 me_b1, 1 me_b2, 2 ue_b1, 3 ue_b2, 4 cm_vb, 5 2*cm_ob,
    # 6 mg_b, 7 ug_b, 8 qb0', 9 qb1', 10 kb0, 11 kb1
    i_bc = din("bc", [P, 12])
    i_wq = din("wq", [L, P, P])            # qw / sqrt(D)
    i_wk = din("wk", [L, P, P])            # rel_mean[:,None] * kw
    i_wv = din("wv", [L, P, P])
    i_vbb = din("vbb", [P, L * P])         # vb broadcast tiles
    i_wla = din("wla", [384, P])
    i_lab = din("lab", [1, P])

    o_fin = nc.dram_tensor("o_fin", [RPD, P], f32, kind="ExternalOutput")
    o_mmT = nc.dram_tensor("o_mmT", [P, RPD], f32, kind="ExternalOutput")

    # ---- internal DRAM ----
    e_locs = [nc.dram_tensor(f"e_loc{l}", [RPD, P], f32) for l in range(L)]
    e_full = [nc.dram_tensor(f"e_full{l}", [NPAD, P], f32, addr_space="Shared")
              for l in range(L)]
    sT_locs = [nc.dram_tensor(f"sT_loc{l}", [P, RPD], f32) for l in range(L)]
    sT_fulls = [nc.dram_tensor(f"sT_full{l}", [NCORES, P, RPD], f32,
                               addr_space="Shared") for l in range(L)]

    ident_np = np.eye(P, dtype=np.float32)
    c_ident = nc.inline_tensor(ident_np, name="c_ident")
    c_ones = nc.inline_tensor(np.ones((1, P), np.float32), name="c_ones")

    RG = [list(range(NCORES))]

    with TileContext(nc) as tc:
        with (
            tc.tile_pool(name="wts", bufs=1) as wts,
            tc.tile_pool(name="big", bufs=1) as bigp,
            tc.tile_pool(name="sc2", bufs=1) as sc2,
            tc.tile_pool(name="str", bufs=3) as strm,
            tc.tile_pool(name="sm", bufs=3) as smp,
            tc.tile_pool(name="ps", bufs=2, space="PSUM") as ps,
            tc.tile_pool(name="psT", bufs=2, space="PSUM") as psT,
            tc.tile_pool(name="psacc", bufs=2, space="PSUM") as psacc,
        ):
            # ---------- persistent SBUF ----------
            ident = wts.tile([P, P], f32)
            nc.sync.dma_start(ident[:], c_ident.ap())
            ones1 = wts.tile([1, P], f32)
            nc.sync.dma_start(ones1[:], c_ones.ap())
            bc = wts.tile([P, 12], f32)
            nc.sync.dma_start(bc[:], i_bc.ap())
            vbb = wts.tile([P, L * P], f32)
            nc.sync.dma_start(vbb[:], i_vbb.ap())
            wq = [wts.tile([P, P], f32, tag=f"wq{l}", name=f"wq{l}") for l in range(L)]
            wk = [wts.tile([P, P], f32, tag=f"wk{l}", name=f"wk{l}") for l in range(L)]
            wv = [wts.tile([P, P], f32, tag=f"wv{l}", name=f"wv{l}") for l in range(L)]
            for l in range(L):
                nc.sync.dma_start(wq[l][:], i_wq.ap()[l])
                nc.sync.dma_start(wk[l][:], i_wk.ap()[l])
                nc.sync.dma_start(wv[l][:], i_wv.ap()[l])

            eT = [bigp.tile([P, RPD], f32, tag=f"eT{l}", name=f"eT{l}") for l in range(L + 1)]
            qnT = bigp.tile([P, RPD], f32)
            knT = bigp.tile([P, NPAD], f32)
            vn_sb = bigp.tile([P, NKT, P], f32)      # 63 tiles [128n,128d]
            u_scr = bigp.tile([P, 2000], f32)
            den16 = bigp.tile([P, NCH], f32)
            bf16 = mybir.dt.bfloat16

            HC = RPD // 512  # 2 chunks of 512

            def mm_chain(out_sb, lhsT_tiles, rhs_sb, bias_col, act, scale=1.0):
                """out_sb[:, h*512:] = act(sum_k lhsT_k.T @ rhs_k + bias)."""
                for h in range(HC):
                    pst = ps.tile([P, 512], f32, tag="gen")
                    nk = len(lhsT_tiles)
                    for k, (lt, rt) in enumerate(zip(lhsT_tiles, rhs_sb)):
                        nc.tensor.matmul(pst[:], lt, rt[:, h * 512:(h + 1) * 512],
                                         start=(k == 0), stop=(k == nk - 1))
                    nc.scalar.activation(out_sb[:, h * 512:(h + 1) * 512], pst[:],
                                         act, bias=bias_col, scale=scale)

            # ---------- prologue ----------
            with (
                tc.tile_pool(name="proW", bufs=1) as proW,
                tc.tile_pool(name="proB", bufs=7) as proB,
            ):
                def chain_dram(out_sb, lhsT_tiles, rhs_dram, bias_col, act,
                               scale=1.0):
                    nk = len(lhsT_tiles)
                    for h in range(HC):
                        pst = ps.tile([P, 512], f32, tag="gen")
                        for k in range(nk):
                            rt = strm.tile([P, 512], f32, tag="pstr",
                                           name=f"pstr_{h}_{k}")
                            nc.sync.dma_start(
                                rt[:],
                                rhs_dram.ap()[k * P:(k + 1) * P,
                                              h * 512:(h + 1) * 512])
                            nc.tensor.matmul(pst[:], lhsT_tiles[k], rt[:],
                                             start=(k == 0), stop=(k == nk - 1))
                        nc.scalar.activation(out_sb[:, h * 512:(h + 1) * 512],
                                             pst[:], act, bias=bias_col,
                                             scale=scale)

                wme1 = [proW.tile([P, P], f32, tag=f"wme1_{k}",
                                  name=f"wme1_{k}") for k in range(6)]
                for k in range(6):
                    nc.sync.dma_start(wme1[k][:],
                                      i_wme1.ap()[k * P:(k + 1) * P, :])
                wme2 = proW.tile([P, P], f32)
                nc.sync.dma_start(wme2[:], i_wme2.ap())
                wue1 = [proW.tile([P, P], f32, tag=f"wue1_{k}",
                                  name=f"wue1_{k}") for k in range(4)]
                for k in range(4):
                    nc.sync.dma_start(wue1[k][:],
                                      i_wue1.ap()[k * P:(k + 1) * P, :])
                wue2 = proW.tile([P, P], f32)
                nc.sync.dma_start(wue2[:], i_wue2.ap())
                wcmv = proW.tile([P, P], f32)
                nc.sync.dma_start(wcmv[:], i_wcmv.ap())
                wcmo = proW.tile([P, P], f32)
                nc.sync.dma_start(wcmo[:], i_wcmo.ap())
                wmg = [proW.tile([P, P], f32, tag=f"wmg{k}", name=f"wmg{k}")
                       for k in range(2)]
                wug = [proW.tile([P, P], f32, tag=f"wug{k}", name=f"wug{k}")
                       for k in range(2)]
                for k in range(2):
                    nc.sync.dma_start(wmg[k][:],
                                      i_wmg.ap()[k * P:(k + 1) * P, :])
                    nc.sync.dma_start(wug[k][:],
                                      i_wug.ap()[k * P:(k + 1) * P, :])

                h1 = proB.tile([P, RPD], f32, tag="pbig")
                chain_dram(h1, [w[:] for w in wme1], i_mfT, bc[:, 0:1], AF.Relu)
                mmIT = proB.tile([P, RPD], f32, tag="pbig")
                mm_chain(mmIT, [wme2[:]], [h1[:]], bc[:, 1:2], AF.Relu)
                vT = proB.tile([P, RPD], f32, tag="pbig")
                mm_chain(vT, [wcmv[:]], [mmIT[:]], bc[:, 4:5], AF.Identity)
                mmI2T = proB.tile([P, RPD], f32, tag="pbig")
                mm_chain(mmI2T, [wcmo[:]], [vT[:]], bc[:, 5:6], AF.Identity,
                         scale=2.0)
                nc.sync.dma_start(o_mmT.ap(), mmI2T[:])

                h1u = proB.tile([P, RPD], f32, tag="pbig")
                chain_dram(h1u, [w[:] for w in wue1], i_ufT, bc[:, 2:3],
                           AF.Relu)
                mmUT = proB.tile([P, RPD], f32, tag="pbig")
                mm_chain(mmUT, [wue2[:]], [h1u[:]], bc[:, 3:4], AF.Relu)

                bT = proB.tile([P, RPD], f32, tag="pbig")
                nc.sync.dma_start(bT[:], i_bT.ap())

                # item path
                s1 = proB.tile([P, RPD], f32, tag="pbig")
                mm_chain(s1, [wmg[0][:], wmg[1][:]], [bT[:], mmI2T[:]],
                         bc[:, 6:7], AF.Sigmoid)
                mgT = proB.tile([P, RPD], f32, tag="pbig")
                for h in range(HC):
                    sl = slice(h * 512, (h + 1) * 512)
                    nc.scalar.activation(mgT[:, sl], s1[:, sl], AF.Sigmoid,
                                         scale=2.0)
                mi = proB.tile([P, RPD], f32, tag="pbig")
                nc.sync.dma_start(mi[:], i_mi.ap())
                tA = proB.tile([P, RPD], f32, tag="pbig")
                nc.vector.tensor_sub(tA[:], bT[:], mmI2T[:])
                nc.vector.tensor_mul(tA[:], mgT[:], tA[:])
                nc.vector.tensor_add(tA[:], tA[:], mmI2T[:])
                nc.vector.scalar_tensor_tensor(
                    out=tA[:], in0=bT[:], scalar=-0.9, in1=tA[:],
                    op0=OP.mult, op1=OP.add)
                nc.vector.tensor_mul(tA[:], mi[:], tA[:])

                # user path
                s1b = proB.tile([P, RPD], f32, tag="pbig")
                mm_chain(s1b, [wug[0][:], wug[1][:]], [bT[:], mmUT[:]],
                         bc[:, 7:8], AF.Sigmoid)
                ugT = proB.tile([P, RPD], f32, tag="pbig")
                for h in range(HC):
                    sl = slice(h * 512, (h + 1) * 512)
                    nc.scalar.activation(ugT[:, sl], s1b[:, sl], AF.Sigmoid,
                                         scale=2.0)
                mu = proB.tile([P, RPD], f32, tag="pbig")
                nc.sync.dma_start(mu[:], i_mu.ap())
                tB = proB.tile([P, RPD], f32, tag="pbig")
                nc.vector.tensor_sub(tB[:], bT[:], mmUT[:])
                nc.vector.tensor_mul(tB[:], ugT[:], tB[:])
                nc.vector.tensor_add(tB[:], tB[:], mmUT[:])
                nc.vector.scalar_tensor_tensor(
                    out=tB[:], in0=bT[:], scalar=-0.9, in1=tB[:],
                    op0=OP.mult, op1=OP.add)
                nc.vector.tensor_mul(tB[:], mu[:], tB[:])

                e0T = eT[0]
                nc.vector.tensor_add(tA[:], tA[:], bT[:])
                nc.vector.tensor_add(e0T[:], tA[:], tB[:])

                for mt in range(MT):
                    pt = psT.tile([P, P], f32, tag="tp")
                    nc.tensor.transpose(pt[:], e0T[:, mt * P:(mt + 1) * P],
                                        ident[:])
                    e0n = smp.tile([P, P], f32, tag="blk")
                    nc.scalar.activation(e0n[:], pt[:], AF.Copy)
                    nc.sync.dma_start(e_locs[0].ap()[mt * P:(mt + 1) * P, :],
                                      e0n[:])
                nc.gpsimd.collective_compute(
                    "AllGather", OP.bypass, replica_groups=RG,
                    ins=[e_locs[0][:]], outs=[e_full[0][:]])

            # ---------- GNN layers ----------
            for l in range(L):
                # A) side^T = (A_dev @ e)^T : accumulate over 64 n-tiles
                psA = [psacc.tile([P, 512], f32, tag="acc", name=f"psA{h}")
                       for h in range(HC)]
                for k in range(AKT):
                    eL = strm.tile([P, P], f32, tag="eL")
                    nc.sync.dma_start(eL[:],
                                      e_full[l].ap()[k * P:(k + 1) * P, :])
                    for h in range(HC):
                        aTc = strm.tile([P, 512], f32, tag="aT")
                        nc.sync.dma_start(
                            aTc[:],
                            i_at.ap()[k * P:(k + 1) * P, h * 512:(h + 1) * 512])
                        nc.tensor.matmul(psA[h][:], eL[:], aTc[:],
                                         start=(k == 0), stop=(k == AKT - 1))
                for h in range(HC):
                    sA = strm.tile([P, 512], f32, tag="sA")
                    nc.scalar.activation(sA[:], psA[h][:], AF.Copy)
                    nc.sync.dma_start(sT_locs[l].ap()[:, h * 512:(h + 1) * 512],
                                      sA[:])
                nc.gpsimd.collective_compute(
                    "AllGather", OP.bypass, replica_groups=RG,
                    ins=[sT_locs[l][:]], outs=[sT_fulls[l][:]])

                # B) qn^T = wq.T @ e^T + qb
                mm_chain(qnT, [wq[l][:]], [eT[l][:]], bc[:, 8 + l:9 + l],
                         AF.Identity)

                # C) kn^T chunks + vn tiles from gathered side^T
                for c in range(NCH):
                    r, hh = c // 2, c % 2
                    sch = smp.tile([P, 512], f32, tag="sch")
                    nc.sync.dma_start(
                        sch[:], sT_fulls[l].ap()[r][:, hh * 512:(hh + 1) * 512])
                    psK = ps.tile([P, 512], f32, tag="gen")
                    nc.tensor.matmul(psK[:], wk[l][:], sch[:],
                                     start=True, stop=True)
                    nc.scalar.activation(knT[:, c * 512:(c + 1) * 512],
                                         psK[:], AF.Identity,
                                         bias=bc[:, 10 + l:11 + l])
                    for t4 in range(4):
                        nt = 4 * c + t4
                        if nt >= NKT:
                            continue
                        psV = ps.tile([P, P], f32, tag="gen")
                        nc.tensor.matmul(psV[:],
                                         sch[:, t4 * P:(t4 + 1) * P], wv[l][:],
                                         start=True, stop=True)
                        if l == 0:
                            nc.scalar.activation(vn_sb[:, nt, :], psV[:],
                                                 AF.Copy)
                        else:
                            vn16 = vn_sb[:].bitcast(bf16)
                            nc.scalar.activation(vn16[:, nt, 0:P], psV[:],
                                                 AF.Copy)

                # D) scores + topk-masked dense aggregation, per m-tile
                sc_chunks = NCH  # 16 chunks of 500
                for mt in range(MT):
                    sc = sc2.tile([P, N], f32, tag="sc")
                    cand = smp.tile([P, P], f32, tag="cand")
                    for c in range(sc_chunks):
                        psS = ps.tile([P, CH], f32, tag="psS")
                        nc.tensor.matmul(
                            psS[:], qnT[:, mt * P:(mt + 1) * P],
                            knT[:, c * CH:(c + 1) * CH],
                            start=True, stop=True)
                        nc.scalar.activation(sc[:, c * CH:(c + 1) * CH],
                                             psS[:], AF.Copy)
                        nc.vector.max(cand[:, c * 8:c * 8 + 8],
                                      sc[:, c * CH:(c + 1) * CH])
                    t8a = smp.tile([P, 8], f32, tag="t8a")
                    nc.vector.max(t8a[:], cand[:])
                    candr = smp.tile([P, P], f32, tag="candr")
                    nc.vector.match_replace(candr[:], t8a[:], cand[:], -BIG)
                    t8b = smp.tile([P, 8], f32, tag="t8b")
                    nc.vector.max(t8b[:], candr[:])
                    nrm = smp.tile([P, 1], f32, tag="nrm")
                    nc.vector.tensor_scalar(out=nrm[:], in0=t8a[:, 0:1],
                                            scalar1=-1.0, scalar2=None,
                                            op0=OP.mult)
                    # per chunk: u=(s-t16)*BIG; u=min(u,0)+s; s=exp(u-rowmax)
                    for c in range(4):
                        cs = slice(c * 2000, (c + 1) * 2000)
                        nc.vector.tensor_scalar(
                            out=u_scr[:], in0=sc[:, cs], scalar1=t8b[:, 7:8],
                            scalar2=BIG, op0=OP.subtract, op1=OP.mult)
                        nc.vector.scalar_tensor_tensor(
                            out=u_scr[:], in0=u_scr[:], scalar=0.0,
                            in1=sc[:, cs], op0=OP.min, op1=OP.add)
                        nc.scalar.activation(sc[:, cs], u_scr[:], AF.Exp,
                                             bias=nrm[:],
                                             accum_out=den16[:, c:c + 1])
                    den = smp.tile([P, 1], f32, tag="den")
                    nc.vector.tensor_reduce(out=den[:], in_=den16[:, 0:4],
                                            axis=mybir.AxisListType.X,
                                            op=OP.add)
                    rden = smp.tile([P, 1], f32, tag="rden")
                    nc.vector.reciprocal(rden[:], den[:])
                    # agg = (w @ vn) * rden + vb ; relu
                    psG = psacc.tile([P, 512], f32, tag="acc")
                    if l == 0:
                        for k in range(NKT):
                            kw_ = P if k < NKT - 1 else N - (NKT - 1) * P
                            pt = psT.tile([P, P], f32, tag="tp")
                            nc.tensor.transpose(pt[:kw_, :],
                                                sc[:, k * P:k * P + kw_],
                                                ident[:])
                            wTt = strm.tile([P, P], f32, tag="wTt")
                            nc.scalar.activation(wTt[:kw_, :], pt[:kw_, :],
                                                 AF.Copy)
                            nc.tensor.matmul(psG[:, 0:P], wTt[:kw_, :],
                                             vn_sb[:kw_, k, :],
                                             start=(k == 0),
                                             stop=(k == NKT - 1))
                    else:
                        sc16 = sc[:].bitcast(bf16)
                        for c4 in range(4):
                            nc.scalar.activation(
                                sc16[:, c4 * 2000:(c4 + 1) * 2000],
                                sc[:, c4 * 2000:(c4 + 1) * 2000], AF.Copy)
                        vn16 = vn_sb[:].bitcast(bf16)
                        for k in range(NKT):
                            kw_ = P if k < NKT - 1 else N - (NKT - 1) * P
                            wT16 = strm.tile([P, P], bf16, tag="wT16")
                            nc.sync.dma_start_transpose(
                                wT16[:], sc16[:, k * P:(k + 1) * P])
                            nc.tensor.matmul(psG[:, 0:P], wT16[:kw_, :],
                                             vn16[:kw_, k, 0:P],
                                             start=(k == 0),
                                             stop=(k == NKT - 1))
                    agg = smp.tile([P, P], f32, tag="blk")
                    nc.scalar.activation(agg[:], psG[:, 0:P], AF.Copy,
                                         scale=rden[:])
                    nc.vector.tensor_add(agg[:], agg[:],
                                         vbb[:, l * P:(l + 1) * P])
                    enx = smp.tile([P, P], f32, tag="blk")
                    nc.scalar.activation(enx[:], agg[:], AF.Relu)
                    if l == 0:
                        nc.sync.dma_start(
                            e_locs[1].ap()[mt * P:(mt + 1) * P, :], enx[:])
                    # e_next^T tile
                    pte = psT.tile([P, P], f32, tag="tp")
                    nc.tensor.transpose(pte[:], enx[:], ident[:])
                    nc.scalar.activation(eT[l + 1][:, mt * P:(mt + 1) * P],
                                         pte[:], AF.Copy)
                if l == 0:
                    nc.gpsimd.collective_compute(
                        "AllGather", OP.bypass, replica_groups=RG,
                        ins=[e_locs[1][:]], outs=[e_full[1][:]])

            # ---------- epilogue: final = cat(e0,e1,e2)@la_w + la_b; norm ----
            wla = [wts.tile([P, P], f32, tag=f"wla{k}", name=f"wla{k}") for k in range(3)]
            for k in range(3):
                nc.sync.dma_start(wla[k][:], i_wla.ap()[k * P:(k + 1) * P, :])
            lab = wts.tile([1, P], f32)
            nc.sync.dma_start(lab[:], i_lab.ap())
            for mt in range(MT):
                psF = ps.tile([P, P], f32, tag="gen")
                for k in range(3):
                    nc.tensor.matmul(psF[:], eT[k][:, mt * P:(mt + 1) * P],
                                     wla[k][:], start=(k == 0), stop=False)
                nc.tensor.matmul(psF[:], ones1[:], lab[:],
                                 start=False, stop=True)
                zsb = smp.tile([P, P], f32, tag="blk")
                nc.scalar.activation(zsb[:], psF[:], AF.Copy)
                junk = smp.tile([P, P], f32, tag="blk")
                ssq = smp.tile([P, 1], f32, tag="ssq")
                nc.vector.scalar_tensor_tensor(
                    out=junk[:], in0=zsb[:], scalar=0.0, in1=zsb[:],
                    op0=OP.add, op1=OP.mult, accum_out=ssq[:])
                nrm2 = smp.tile([P, 1], f32, tag="nrm2")
                nc.scalar.activation(nrm2[:], ssq[:], AF.Sqrt)
                nc.vector.tensor_scalar_max(nrm2[:], nrm2[:], 1e-12)
                rn = smp.tile([P, 1], f32, tag="rn")
                nc.vector.reciprocal(rn[:], nrm2[:])
                fin = smp.tile([P, P], f32, tag="blk")
                nc.scalar.activation(fin[:], zsb[:], AF.Copy, scale=rn[:])
                nc.sync.dma_start(o_fin.ap()[mt * P:(mt + 1) * P, :], fin[:])

    nc.compile()
    return nc


def _host_prep(inputs):
    """Build per-device input maps."""
    f = np.float32
    uw = inputs["user_w"].astype(f)
    iw = inputs["item_w"].astype(f)
    ew = inputs["ent_w"].astype(f)
    base = np.zeros((NPAD, D), f)
    base[0:N_USERS] = uw
    base[N_USERS:N_USERS + N_ITEMS] = iw
    base[N_USERS + N_ITEMS:N] = ew
    mask_u = np.zeros(NPAD, f)
    mask_u[0:N_USERS] = 1.0
    mask_i = np.zeros(NPAD, f)
    mask_i[N_USERS:N_USERS + N_ITEMS] = 1.0

    mfTp = np.zeros((768, NPAD), f)
    mfTp[:, N_USERS:N_USERS + N_ITEMS] = inputs["mfeat"].astype(f).T
    ufTp = np.zeros((512, NPAD), f)
    ufTp[:, 0:N_USERS] = inputs["ufeat"].astype(f).T

    A = np.zeros((NPAD, NPAD), f)
    np.add.at(A, (inputs["adj_row"].astype(np.int64),
                  inputs["adj_col"].astype(np.int64)),
              inputs["adj_data"].astype(f))

    rel_mean = inputs["rel_w"].astype(f).mean(axis=0)
    sq = f(1.0) / np.sqrt(f(D))
    wq = (inputs["gnn_qw"].astype(f) * sq)
    wk = (rel_mean[None, :, None] * inputs["gnn_kw"].astype(f))
    wv = inputs["gnn_vw"].astype(f)
    qb = inputs["gnn_qb"].astype(f) * sq
    kb = inputs["gnn_kb"].astype(f)
    vb = inputs["gnn_vb"].astype(f)

    bc = np.zeros((P, 12), f)
    bc[:, 0] = inputs["me_b1"]
    bc[:, 1] = inputs["me_b2"]
    bc[:, 2] = inputs["ue_b1"]
    bc[:, 3] = inputs["ue_b2"]
    bc[:, 4] = inputs["cm_vb"]
    bc[:, 5] = 2.0 * inputs["cm_ob"].astype(f)
    bc[:, 6] = inputs["mg_b"]
    bc[:, 7] = inputs["ug_b"]
    bc[:, 8] = qb[0]
    bc[:, 9] = qb[1]
    bc[:, 10] = kb[0]
    bc[:, 11] = kb[1]

    vbb = np.zeros((P, L * P), f)
    for l in range(L):
        vbb[:, l * P:(l + 1) * P] = np.tile(vb[l][None, :], (P, 1))

    in_maps = []
    for dv in range(NCORES):
        rows = slice(dv * RPD, (dv + 1) * RPD)
        in_maps.append(dict(
            at=np.ascontiguousarray(A[rows, :].T),
            mfT=np.ascontiguousarray(mfTp[:, rows]),
            ufT=np.ascontiguousarray(ufTp[:, rows]),
            bT=np.ascontiguousarray(base[rows, :].T),
            mi=np.tile(mask_i[rows][None, :], (P, 1)),
            mu=np.tile(mask_u[rows][None, :], (P, 1)),
            wme1=inputs["me_w1"].astype(f), wme2=inputs["me_w2"].astype(f),
            wue1=inputs["ue_w1"].astype(f), wue2=inputs["ue_w2"].astype(f),
            wcmv=inputs["cm_vw"].astype(f), wcmo=inputs["cm_ow"].astype(f),
            wmg=inputs["mg_w"].astype(f), wug=inputs["ug_w"].astype(f),
            bc=bc, wq=wq, wk=wk, wv=wv, vbb=vbb,
            wla=inputs["la_w"].astype(f),
            lab=inputs["la_b"].astype(f)[None, :],
        ))
    return in_maps


def kernel(**inputs):
    from concourse.bass_utils import run_bass_kernel_spmd

    if "nc" not in _PROGRAM_CACHE:
        _PROGRAM_CACHE["nc"] = _build_program()
    nc = _PROGRAM_CACHE["nc"]

    in_maps = _host_prep(inputs)
    _PROGRAM_CACHE["last_in_maps"] = in_maps
    r = run_bass_kernel_spmd(nc, in_maps, list(range(NCORES)))
    _PROGRAM_CACHE["last_result"] = r
    res = r.results

    fin = np.concatenate([res[dv]["o_fin"] for dv in range(NCORES)], axis=0)
    mmT = np.concatenate([res[dv]["o_mmT"] for dv in range(NCORES)], axis=1)
    user_out = fin[0:N_USERS]
    item_out = fin[N_USERS:N_USERS + N_ITEMS]
    mm_item = np.ascontiguousarray(mmT[:, N_USERS:N_USERS + N_ITEMS].T)
    item_w = np.asarray(inputs["item_w"], np.float32).copy()
    return (user_out, item_out, item_w, mm_item)
